# revision 6
# baseline (speedup 1.0000x reference)
"""LSTM encoder kernel for Trainium2 (Bass/Tile), data-parallel over batch on 8 cores.

Math (per core, batch shard B=256), sigmoid-only reparametrization:
  z = Wcat @ [hh_{t-1} ; x_t]   with hh = h/2, cc = c/2; hh-rows of Wcat are
  scaled by 2 and g-gate columns by 2, so a single sigmoid over all 128 gate
  rows yields S_g = sigmoid(2 z_g), i.e. tanh(z_g) = 2 S_g - 1.
  Gate row order [g, i, f, o]:
    S    = sigmoid(z + b)             (ACT, one op, 128 rows: Sg2@0, Si@32, Sf@64, So@96)
    t1   = S_g - 0.5                  (DVE, @0 -> @32; = tanh(z_g)/2)
    u    = t1 * S_i                   (DVE, @32/@32 -> @64; = i*g/2)
    v    = S_f * cc_{t-1}             (Pool, @64/@64 -> @64; = f*c/2)
    cc   = u + v                      (DVE, @64/@64 -> @64; = c_new/2)
    ts   = sigmoid(4*cc)              (ACT, @64 -> @96; = sigmoid(2 c_new))
    hh   = (ts - 0.5) * S_o           (DVE STT, @96/@96 -> bf16 rhs slot @0; = h/2)
  Host multiplies the stored hh history by 2 to recover h.

Precision: weights/x/hh in bf16 (PE 1 cycle/row, single LDWEIGHTS), cc and all
elementwise in fp32. Weights are loaded into the PE array ONCE via a standalone
ldweights; per-step matmuls are emitted with ldweights=False.

Scheduling: the Tile list-scheduler is steered with tile_set_cur_wait ticks so
the two batch blocks run half a step out of phase (without this the greedy
scheduler collapses them in-phase and the other block's ACT ops land on the
recurrence critical path).
"""

import numpy as np
import ml_dtypes
from contextlib import ExitStack

import concourse.bass as bass
import concourse.tile as tile
from concourse import bacc, mybir
from concourse.bass_utils import run_bass_kernel_spmd

T_FULL = 512
B_FULL = 2048
IN = 10
H = 32
G = 4 * H          # 128 gate rows
K = IN + H         # 42 contraction rows of the combined matmul
NCORES = 8
B = B_FULL // NCORES  # 256 batch per core

NB = 2          # batch sub-blocks per core (latency pipelining)
FD = B // NB    # free-dim per block
TC = 16         # timesteps per SBUF chunk
HALF_NS = 1400  # scheduler-sim stagger per half-iteration (order control only)

BF16 = mybir.dt.bfloat16
F32 = mybir.dt.float32
SIG = mybir.ActivationFunctionType.Sigmoid
MULT = mybir.AluOpType.mult
ADD = mybir.AluOpType.add
SUB = mybir.AluOpType.subtract

_CACHE = {}


def _mm_noldw(nc, out, lhsT, rhs):
    """MATMUL that reuses the PE-resident weights (no LDWEIGHTS emitted)."""
    te = nc.tensor
    ifmap_ap = te.lower_ap(rhs.opt({0}), opt=False)
    weights_ap = te.lower_ap(lhsT.opt({0}), opt=False, for_matmul_weights=True)
    out_ap = te.lower_ap(out)
    return te.add_instruction(
        mybir.InstMatmult(
            name=te.bass.get_next_instruction_name(),
            replication_resolution=0,
            replication_shift_amnt=0,
            replication_num_rows=0,
            start_tensor_calc=True,
            stop_tensor_calc=True,
            ins=[ifmap_ap, weights_ap],
            outs=[out_ap],
            perf_mode=None,
            is_transpose=None,
            ifmap_quant_offset=None,
            weights_quant_offset=None,
            bass_skip_group_check=False,
            tile_position=None,
            tile_size=None,
            ldweights=False,
        )
    )


def _build(t_total=T_FULL, tc=TC, nb=NB):
    fd = B // nb
    nchunk = t_total // tc
    nc = bacc.Bacc(trn_type="TRN2", debug=False, target_bir_lowering=False)

    xT = nc.dram_tensor("xT", [t_total, IN, B], BF16, kind="ExternalInput").ap()
    wcat = nc.dram_tensor("wcat", [K, G], BF16, kind="ExternalInput").ap()
    bg = nc.dram_tensor("bg", [G, 1], F32, kind="ExternalInput").ap()
    hout = nc.dram_tensor("hout", [t_total, H, B], BF16, kind="ExternalOutput").ap()

    with tile.TileContext(nc) as tc_, ExitStack() as ctx:
        const = ctx.enter_context(tc_.tile_pool(name="const", bufs=1))
        xpool = ctx.enter_context(tc_.tile_pool(name="xpool", bufs=3))
        spool = ctx.enter_context(tc_.tile_pool(name="spool", bufs=4))
        taupool = ctx.enter_context(tc_.tile_pool(name="taupool", bufs=4))
        cpool = ctx.enter_context(tc_.tile_pool(name="cpool", bufs=4))
        tpool = ctx.enter_context(tc_.tile_pool(name="tpool", bufs=8))
        pspool = ctx.enter_context(tc_.tile_pool(name="pspool", bufs=4, space="PSUM"))

        w_t = const.tile([K, G], BF16)
        nc.sync.dma_start(w_t[:], wcat)
        bg_t = const.tile([G, 1], F32)
        nc.sync.dma_start(bg_t[:], bg)

        # one-time weight load; every step's matmul reuses the resident array
        nc.tensor.ldweights(w_t[:])

        # rhs chunk tiles: [K, tc*B] bf16; rows 0:H = hh slots, rows H:K = x slots
        chunk_tiles = {}

        def get_chunk(ch):
            if ch not in chunk_tiles:
                t = xpool.tile([K, tc * B], BF16, name="rhs", tag="rhs")
                if ch < nchunk:
                    nc.sync.dma_start(
                        t[H:K].rearrange("p (t b) -> p t b", t=tc),
                        xT[ch * tc:(ch + 1) * tc].rearrange("t p b -> p t b"),
                    )
                chunk_tiles[ch] = t
            return chunk_tiles[ch]

        cur = get_chunk(0)
        # hh_{-1} = 0
        nc.vector.memset(cur[0:H, 0:B], 0.0)

        c_prev = []
        for blk in range(nb):
            c0 = cpool.tile([3 * H, fd], F32, name=f"cc{blk}", tag=f"cc{blk}")
            nc.vector.memset(c0[2 * H:3 * H], 0.0)
            c_prev.append(c0)

        # Phase A(b, s): mm -> sigma_all -> t1 -> u ; v (Pool)
        # Phase B(b, s): cc -> ts -> hh
        state = {}

        def phase_a(blk, s_global):
            ch_, s_ = divmod(s_global, tc)
            col = s_ * B + blk * fd
            rhs = get_chunk(ch_)
            p = pspool.tile([G, fd], F32, name="gates", tag=f"gates{blk}")
            _mm_noldw(nc, p[:], w_t[:], rhs[:, col:col + fd])
            # S = sigmoid(z): Sg2@0, Si@32, Sf@64, So@96
            s_t = spool.tile([G, fd], F32, name="sgm", tag=f"sgm{blk}")
            nc.scalar.activation(s_t[:], p[:], SIG, bias=bg_t[:])
            # t1 = S_g - 0.5 relocated to start 32 (pairs with i)
            t1 = tpool.tile([2 * H, fd], F32, name="t1", tag=f"t1{blk}")
            nc.vector.tensor_scalar(t1[H:2 * H], s_t[0:H], 0.5, None, SUB)
            # v = f * cc_prev at start 64 (Pool, off the DVE chain)
            v = tpool.tile([3 * H, fd], F32, name="v", tag=f"v{blk}")
            nc.gpsimd.tensor_tensor(
                v[2 * H:3 * H], s_t[2 * H:3 * H], c_prev[blk][2 * H:3 * H], MULT)
            # u = t1 * S_i (both at start 32), placed at start 64
            u = tpool.tile([3 * H, fd], F32, name="u", tag=f"u{blk}")
            nc.vector.tensor_tensor(u[2 * H:3 * H], t1[H:2 * H], s_t[H:2 * H],
                                    MULT)
            state[blk] = (s_t, u, v, s_global)

        def phase_b(blk):
            s_t, u, v, s_global = state[blk]
            c_new = cpool.tile([3 * H, fd], F32, name=f"ccn{blk}",
                               tag=f"cc{blk}")
            nc.vector.tensor_tensor(c_new[2 * H:3 * H], u[2 * H:3 * H],
                                    v[2 * H:3 * H], ADD)
            c_prev[blk] = c_new
            # ts = sigmoid(4*cc) relocated to start 96 (pairs with o)
            ts = taupool.tile([G, fd], F32, name="ts", tag=f"ts{blk}")
            nc.scalar.activation(ts[3 * H:4 * H], c_new[2 * H:3 * H],
                                 SIG, scale=4.0)
            ch_, s_ = divmod(s_global + 1, tc)
            col = s_ * B + blk * fd
            hdst = get_chunk(ch_)[0:H, col:col + fd]
            nc.vector.scalar_tensor_tensor(
                hdst, ts[3 * H:4 * H], 0.5, s_t[3 * H:4 * H], SUB, MULT)

        def emit_out(ch):
            cur_, nxt_ = get_chunk(ch), get_chunk(ch + 1)
            nc.sync.dma_start(
                hout[ch * tc:ch * tc + tc - 1].rearrange("t p b -> p t b"),
                cur_[0:H, B:].rearrange("p (t b) -> p t b", t=tc - 1),
            )
            nc.sync.dma_start(hout[ch * tc + tc - 1], nxt_[0:H, 0:B])

        def tick(n):
            tc_.tile_set_cur_wait(n * HALF_NS / 1e6)

        tick(0)
        phase_a(0, 0)
        for s in range(t_total):
            tick(2 * s + 1)
            phase_a(1, s)
            phase_b(0)
            tick(2 * s + 2)
            if s + 1 < t_total:
                phase_a(0, s + 1)
            phase_b(1)
            if s % tc == tc - 1:
                emit_out(s // tc)
    nc.compile()
    return nc


def _prep_weights(W_emb, b_emb, W_ih, W_hh, b_ih, b_hh):
    f8 = lambda a: np.asarray(a, np.float64)
    Wx = f8(W_ih) @ f8(W_emb)                                  # [4H, IN]
    bgv = f8(W_ih) @ f8(b_emb) + f8(b_ih) + f8(b_hh)           # [4H]
    perm = np.r_[2 * H:3 * H, 0:H, H:2 * H, 3 * H:4 * H]       # [g,i,f,o]
    wc = np.concatenate([f8(W_hh)[perm].T, Wx[perm].T], axis=0)  # [K, G]
    wc[0:H, :] *= 2.0           # hh-rows: rhs holds h/2
    wc[:, 0:H] *= 2.0           # g-gate columns: sigmoid(2 z_g)
    bgv = bgv[perm].copy()
    bgv[0:H] *= 2.0
    return (np.ascontiguousarray(wc.astype(ml_dtypes.bfloat16)),
            np.ascontiguousarray(bgv.astype(np.float32).reshape(G, 1)))


def _run(x, W_emb, b_emb, W_ih, W_hh, b_ih, b_hh, trace=False):
    t_total = x.shape[0]
    key = (t_total, TC, NB)
    if key not in _CACHE:
        _CACHE[key] = _build(t_total, TC, NB)
    nc = _CACHE[key]

    wc, bgv = _prep_weights(W_emb, b_emb, W_ih, W_hh, b_ih, b_hh)
    x = np.asarray(x, np.float32)
    in_maps = []
    for c in range(NCORES):
        xs = np.ascontiguousarray(
            x[:, c * B:(c + 1) * B, :].transpose(0, 2, 1)).astype(
                ml_dtypes.bfloat16)  # [T, IN, B] bf16
        in_maps.append({"xT": xs, "wcat": wc, "bg": bgv})

    res = run_bass_kernel_spmd(nc, in_maps, list(range(NCORES)), trace=trace)
    out = np.empty((t_total, B_FULL, H), np.float32)
    for c in range(NCORES):
        out[:, c * B:(c + 1) * B, :] = np.asarray(
            res.results[c]["hout"], np.float32).transpose(0, 2, 1) * 2.0
    return out, res


def kernel(x, W_emb, b_emb, W_ih, W_hh, b_ih, b_hh):
    out, _ = _run(x, W_emb, b_emb, W_ih, W_hh, b_ih, b_hh, trace=False)
    return out


# revision 7
# speedup vs baseline: 1.0001x; 1.0001x over previous
"""LSTM encoder kernel for Trainium2 (Bass/Tile), data-parallel over batch on 8 cores.

Math (per core, batch shard B=256), sigmoid-only reparametrization:
  z = Wcat @ [hh_{t-1} ; x_t]   with hh = h/2, cc = c/2; hh-rows of Wcat are
  scaled by 2 and g-gate columns by 2, so a single sigmoid over all 128 gate
  rows yields S_g = sigmoid(2 z_g), i.e. tanh(z_g) = 2 S_g - 1.
  Gate row order [g, i, f, o]:
    S    = sigmoid(z + b)             (ACT, one op, 128 rows: Sg2@0, Si@32, Sf@64, So@96)
    t1   = S_g - 0.5                  (DVE, @0 -> @32; = tanh(z_g)/2)
    u    = t1 * S_i                   (DVE, @32/@32 -> @64; = i*g/2)
    v    = S_f * cc_{t-1}             (Pool, @64/@64 -> @64; = f*c/2)
    cc   = u + v                      (DVE, @64/@64 -> @64; = c_new/2)
    ts   = sigmoid(4*cc)              (ACT, @64 -> @96; = sigmoid(2 c_new))
    hh   = (ts - 0.5) * S_o           (DVE STT, @96/@96 -> bf16 rhs slot @0; = h/2)
  Host multiplies the stored hh history by 2 to recover h.

Precision: weights/x/hh in bf16 (PE 1 cycle/row, single LDWEIGHTS), cc and all
elementwise in fp32. Weights are loaded into the PE array ONCE via a standalone
ldweights; per-step matmuls are emitted with ldweights=False.

Scheduling: the Tile list-scheduler is steered with tile_set_cur_wait ticks so
the two batch blocks run half a step out of phase (without this the greedy
scheduler collapses them in-phase and the other block's ACT ops land on the
recurrence critical path).
"""

import numpy as np
import ml_dtypes
from contextlib import ExitStack

import concourse.bass as bass
import concourse.tile as tile
from concourse import bacc, mybir
from concourse.bass_utils import run_bass_kernel_spmd

T_FULL = 512
B_FULL = 2048
IN = 10
H = 32
G = 4 * H          # 128 gate rows
K = IN + H         # 42 contraction rows of the combined matmul
NCORES = 8
B = B_FULL // NCORES  # 256 batch per core

NB = 2          # batch sub-blocks per core (latency pipelining)
FD = B // NB    # free-dim per block
TC = 16         # timesteps per SBUF chunk
HALF_NS = 1400  # scheduler-sim stagger per half-iteration (order control only)

BF16 = mybir.dt.bfloat16
F32 = mybir.dt.float32
SIG = mybir.ActivationFunctionType.Sigmoid
MULT = mybir.AluOpType.mult
ADD = mybir.AluOpType.add
SUB = mybir.AluOpType.subtract

_CACHE = {}


def _mm_noldw(nc, out, lhsT, rhs):
    """MATMUL that reuses the PE-resident weights (no LDWEIGHTS emitted)."""
    te = nc.tensor
    ifmap_ap = te.lower_ap(rhs.opt({0}), opt=False)
    weights_ap = te.lower_ap(lhsT.opt({0}), opt=False, for_matmul_weights=True)
    out_ap = te.lower_ap(out)
    return te.add_instruction(
        mybir.InstMatmult(
            name=te.bass.get_next_instruction_name(),
            replication_resolution=0,
            replication_shift_amnt=0,
            replication_num_rows=0,
            start_tensor_calc=True,
            stop_tensor_calc=True,
            ins=[ifmap_ap, weights_ap],
            outs=[out_ap],
            perf_mode=None,
            is_transpose=None,
            ifmap_quant_offset=None,
            weights_quant_offset=None,
            bass_skip_group_check=False,
            tile_position=None,
            tile_size=None,
            ldweights=False,
        )
    )


def _build(t_total=T_FULL, tc=TC, nb=NB):
    fd = B // nb
    nchunk = t_total // tc
    nc = bacc.Bacc(trn_type="TRN2", debug=False, target_bir_lowering=False)

    xT = nc.dram_tensor("xT", [t_total, IN, B], BF16, kind="ExternalInput").ap()
    wcat = nc.dram_tensor("wcat", [K, G], BF16, kind="ExternalInput").ap()
    bg = nc.dram_tensor("bg", [G, 1], F32, kind="ExternalInput").ap()
    hout = nc.dram_tensor("hout", [t_total, H, B], BF16, kind="ExternalOutput").ap()

    with tile.TileContext(nc) as tc_, ExitStack() as ctx:
        const = ctx.enter_context(tc_.tile_pool(name="const", bufs=1))
        xpool = ctx.enter_context(tc_.tile_pool(name="xpool", bufs=3))
        spool = ctx.enter_context(tc_.tile_pool(name="spool", bufs=4))
        taupool = ctx.enter_context(tc_.tile_pool(name="taupool", bufs=4))
        cpool = ctx.enter_context(tc_.tile_pool(name="cpool", bufs=4))
        tpool = ctx.enter_context(tc_.tile_pool(name="tpool", bufs=8))
        pspool = ctx.enter_context(tc_.tile_pool(name="pspool", bufs=4, space="PSUM"))

        w_t = const.tile([K, G], BF16)
        nc.sync.dma_start(w_t[:], wcat)
        bg_t = const.tile([G, 1], F32)
        nc.sync.dma_start(bg_t[:], bg)

        # one-time weight load; every step's matmul reuses the resident array
        nc.tensor.ldweights(w_t[:])

        # rhs chunk tiles: [K, tc*B] bf16; rows 0:H = hh slots, rows H:K = x slots
        chunk_tiles = {}

        def get_chunk(ch):
            if ch not in chunk_tiles:
                t = xpool.tile([K, tc * B], BF16, name="rhs", tag="rhs")
                if ch < nchunk:
                    nc.sync.dma_start(
                        t[H:K].rearrange("p (t b) -> p t b", t=tc),
                        xT[ch * tc:(ch + 1) * tc].rearrange("t p b -> p t b"),
                    )
                chunk_tiles[ch] = t
            return chunk_tiles[ch]

        cur = get_chunk(0)
        # hh_{-1} = 0
        nc.vector.memset(cur[0:H, 0:B], 0.0)

        c_prev = []
        for blk in range(nb):
            c0 = cpool.tile([3 * H, fd], F32, name=f"cc{blk}", tag=f"cc{blk}")
            nc.vector.memset(c0[2 * H:3 * H], 0.0)
            c_prev.append(c0)

        # Phase A(b, s): mm -> sigma_all -> t1 -> u ; v (Pool)
        # Phase B(b, s): cc -> ts -> hh
        state = {}

        def phase_a(blk, s_global):
            ch_, s_ = divmod(s_global, tc)
            col = s_ * B + blk * fd
            rhs = get_chunk(ch_)
            p = pspool.tile([G, fd], F32, name="gates", tag=f"gates{blk}")
            _mm_noldw(nc, p[:], w_t[:], rhs[:, col:col + fd])
            # S = sigmoid(z): Sg2@0, Si@32, Sf@64, So@96
            s_t = spool.tile([G, fd], F32, name="sgm", tag=f"sgm{blk}")
            nc.scalar.activation(s_t[:], p[:], SIG, bias=bg_t[:])
            # t1 = S_g - 0.5 relocated to start 32 (pairs with i)
            t1 = tpool.tile([2 * H, fd], F32, name="t1", tag=f"t1{blk}")
            nc.vector.tensor_scalar(t1[H:2 * H], s_t[0:H], 0.5, None, SUB)
            # v = f * cc_prev at start 64 (Pool, off the DVE chain)
            v = tpool.tile([3 * H, fd], F32, name="v", tag=f"v{blk}")
            nc.gpsimd.tensor_tensor(
                v[2 * H:3 * H], s_t[2 * H:3 * H], c_prev[blk][2 * H:3 * H], MULT)
            # u = t1 * S_i (both at start 32), placed at start 64
            u = tpool.tile([3 * H, fd], F32, name="u", tag=f"u{blk}")
            nc.vector.tensor_tensor(u[2 * H:3 * H], t1[H:2 * H], s_t[H:2 * H],
                                    MULT)
            state[blk] = (s_t, u, v, s_global)

        def phase_b(blk):
            s_t, u, v, s_global = state[blk]
            c_new = cpool.tile([3 * H, fd], F32, name=f"ccn{blk}",
                               tag=f"cc{blk}")
            nc.vector.tensor_tensor(c_new[2 * H:3 * H], u[2 * H:3 * H],
                                    v[2 * H:3 * H], ADD)
            c_prev[blk] = c_new
            # ts = sigmoid(4*cc) relocated to start 96 (pairs with o)
            ts = taupool.tile([G, fd], F32, name="ts", tag=f"ts{blk}")
            nc.scalar.activation(ts[3 * H:4 * H], c_new[2 * H:3 * H],
                                 SIG, scale=4.0)
            ch_, s_ = divmod(s_global + 1, tc)
            col = s_ * B + blk * fd
            hdst = get_chunk(ch_)[0:H, col:col + fd]
            nc.vector.scalar_tensor_tensor(
                hdst, ts[3 * H:4 * H], 0.5, s_t[3 * H:4 * H], SUB, MULT)

        def emit_out(ch):
            cur_, nxt_ = get_chunk(ch), get_chunk(ch + 1)
            nc.sync.dma_start(
                hout[ch * tc:ch * tc + tc - 1].rearrange("t p b -> p t b"),
                cur_[0:H, B:].rearrange("p (t b) -> p t b", t=tc - 1),
            )
            nc.sync.dma_start(hout[ch * tc + tc - 1], nxt_[0:H, 0:B])

        def tick(n):
            tc_.tile_set_cur_wait(n * HALF_NS / 1e6)

        # Half h: [B-phase of one block, then A-phase of the other]. Issuing B
        # first keeps each block's DVE chain ops from queueing behind the
        # other block's, so the two data cycles overlap cleanly.
        tick(0)
        phase_a(0, 0)
        for s in range(t_total):
            tick(2 * s + 1)
            phase_b(0)
            phase_a(1, s)
            tick(2 * s + 2)
            phase_b(1)
            if s + 1 < t_total:
                phase_a(0, s + 1)
            if s % tc == tc - 1:
                emit_out(s // tc)
    nc.compile()
    return nc


def _prep_weights(W_emb, b_emb, W_ih, W_hh, b_ih, b_hh):
    f8 = lambda a: np.asarray(a, np.float64)
    Wx = f8(W_ih) @ f8(W_emb)                                  # [4H, IN]
    bgv = f8(W_ih) @ f8(b_emb) + f8(b_ih) + f8(b_hh)           # [4H]
    perm = np.r_[2 * H:3 * H, 0:H, H:2 * H, 3 * H:4 * H]       # [g,i,f,o]
    wc = np.concatenate([f8(W_hh)[perm].T, Wx[perm].T], axis=0)  # [K, G]
    wc[0:H, :] *= 2.0           # hh-rows: rhs holds h/2
    wc[:, 0:H] *= 2.0           # g-gate columns: sigmoid(2 z_g)
    bgv = bgv[perm].copy()
    bgv[0:H] *= 2.0
    return (np.ascontiguousarray(wc.astype(ml_dtypes.bfloat16)),
            np.ascontiguousarray(bgv.astype(np.float32).reshape(G, 1)))


def _run(x, W_emb, b_emb, W_ih, W_hh, b_ih, b_hh, trace=False):
    t_total = x.shape[0]
    key = (t_total, TC, NB)
    if key not in _CACHE:
        _CACHE[key] = _build(t_total, TC, NB)
    nc = _CACHE[key]

    wc, bgv = _prep_weights(W_emb, b_emb, W_ih, W_hh, b_ih, b_hh)
    x = np.asarray(x, np.float32)
    in_maps = []
    for c in range(NCORES):
        xs = np.ascontiguousarray(
            x[:, c * B:(c + 1) * B, :].transpose(0, 2, 1)).astype(
                ml_dtypes.bfloat16)  # [T, IN, B] bf16
        in_maps.append({"xT": xs, "wcat": wc, "bg": bgv})

    res = run_bass_kernel_spmd(nc, in_maps, list(range(NCORES)), trace=trace)
    out = np.empty((t_total, B_FULL, H), np.float32)
    for c in range(NCORES):
        out[:, c * B:(c + 1) * B, :] = np.asarray(
            res.results[c]["hout"], np.float32).transpose(0, 2, 1) * 2.0
    return out, res


def kernel(x, W_emb, b_emb, W_ih, W_hh, b_ih, b_hh):
    out, _ = _run(x, W_emb, b_emb, W_ih, W_hh, b_ih, b_hh, trace=False)
    return out


# revision 12
# speedup vs baseline: 1.0723x; 1.0721x over previous
"""LSTM encoder kernel for Trainium2 (Bass/Tile), data-parallel over batch on 8 cores.

Math (per core, batch shard B=256), sigmoid-only reparametrization:
  z = Wcat @ [hh_{t-1} ; x_t]   with hh = h/2, cc = c/2; hh-rows of Wcat are
  scaled by 2 and g-gate columns by 2, so a single sigmoid over all 128 gate
  rows yields S_g = sigmoid(2 z_g), i.e. tanh(z_g) = 2 S_g - 1.
  Gate row order [g, i, f, o]:
    S    = sigmoid(z + b)             (ACT, one op, 128 rows: Sg2@0, Si@32, Sf@64, So@96)
    t1   = S_g - 0.5                  (DVE, @0 -> @32; = tanh(z_g)/2)
    u    = t1 * S_i                   (DVE, @32/@32 -> @64; = i*g/2)
    v    = S_f * cc_{t-1}             (Pool, @64/@64 -> @64; = f*c/2)
    cc   = u + v                      (DVE, @64/@64 -> @64; = c_new/2)
    ts   = sigmoid(4*cc)              (ACT, @64 -> @96; = sigmoid(2 c_new))
    hh   = (ts - 0.5) * S_o           (DVE STT, @96/@96 -> bf16 rhs slot @0; = h/2)
  Host multiplies the stored hh history by 2 to recover h.

Precision: weights/x/hh in bf16 (PE 1 cycle/row, single LDWEIGHTS), cc and all
elementwise in fp32. Weights are loaded into the PE array ONCE via a standalone
ldweights; per-step matmuls are emitted with ldweights=False.

Scheduling: the Tile list-scheduler is steered with tile_set_cur_wait ticks so
the two batch blocks run half a step out of phase (without this the greedy
scheduler collapses them in-phase and the other block's ACT ops land on the
recurrence critical path).
"""

import numpy as np
import ml_dtypes
from contextlib import ExitStack

import concourse.bass as bass
import concourse.tile as tile
from concourse import bacc, mybir
from concourse.bass_utils import run_bass_kernel_spmd

T_FULL = 512
B_FULL = 2048
IN = 10
H = 32
G = 4 * H          # 128 gate rows
K = IN + H         # 42 contraction rows of the combined matmul
NCORES = 8
B = B_FULL // NCORES  # 256 batch per core

NB = 2          # batch sub-blocks per core (latency pipelining)
FD = B // NB    # free-dim per block
TC = 16         # timesteps per SBUF chunk
HALF_NS = 1000  # scheduler-sim stagger per half-iteration (order control only)

BF16 = mybir.dt.bfloat16
F32 = mybir.dt.float32
SIG = mybir.ActivationFunctionType.Sigmoid
MULT = mybir.AluOpType.mult
ADD = mybir.AluOpType.add
SUB = mybir.AluOpType.subtract

_CACHE = {}


def _mm_noldw(nc, out, lhsT, rhs):
    """MATMUL that reuses the PE-resident weights (no LDWEIGHTS emitted)."""
    te = nc.tensor
    ifmap_ap = te.lower_ap(rhs.opt({0}), opt=False)
    weights_ap = te.lower_ap(lhsT.opt({0}), opt=False, for_matmul_weights=True)
    out_ap = te.lower_ap(out)
    return te.add_instruction(
        mybir.InstMatmult(
            name=te.bass.get_next_instruction_name(),
            replication_resolution=0,
            replication_shift_amnt=0,
            replication_num_rows=0,
            start_tensor_calc=True,
            stop_tensor_calc=True,
            ins=[ifmap_ap, weights_ap],
            outs=[out_ap],
            perf_mode=None,
            is_transpose=None,
            ifmap_quant_offset=None,
            weights_quant_offset=None,
            bass_skip_group_check=False,
            tile_position=None,
            tile_size=None,
            ldweights=False,
        )
    )


def _build(t_total=T_FULL, tc=TC, nb=NB):
    fd = B // nb
    nchunk = t_total // tc
    nc = bacc.Bacc(trn_type="TRN2", debug=False, target_bir_lowering=False)

    xT = nc.dram_tensor("xT", [t_total, IN, B], BF16, kind="ExternalInput").ap()
    wcat = nc.dram_tensor("wcat", [K, G], BF16, kind="ExternalInput").ap()
    bg = nc.dram_tensor("bg", [G, 1], F32, kind="ExternalInput").ap()
    hout = nc.dram_tensor("hout", [t_total, H, B], BF16, kind="ExternalOutput").ap()

    with tile.TileContext(nc) as tc_, ExitStack() as ctx:
        const = ctx.enter_context(tc_.tile_pool(name="const", bufs=1))
        xpool = ctx.enter_context(tc_.tile_pool(name="xpool", bufs=3))
        spool = ctx.enter_context(tc_.tile_pool(name="spool", bufs=4))
        taupool = ctx.enter_context(tc_.tile_pool(name="taupool", bufs=4))
        cpool = ctx.enter_context(tc_.tile_pool(name="cpool", bufs=4))
        tpool = ctx.enter_context(tc_.tile_pool(name="tpool", bufs=8))
        pspool = ctx.enter_context(tc_.tile_pool(name="pspool", bufs=4, space="PSUM"))

        w_t = const.tile([K, G], BF16)
        nc.sync.dma_start(w_t[:], wcat)
        bg_t = const.tile([G, 1], F32)
        nc.sync.dma_start(bg_t[:], bg)

        # one-time weight load; every step's matmul reuses the resident array
        nc.tensor.ldweights(w_t[:])

        # rhs chunk tiles: [K, tc*B] bf16; rows 0:H = hh slots, rows H:K = x slots
        chunk_tiles = {}

        def get_chunk(ch):
            if ch not in chunk_tiles:
                t = xpool.tile([K, tc * B], BF16, name="rhs", tag="rhs")
                if ch < nchunk:
                    nc.sync.dma_start(
                        t[H:K].rearrange("p (t b) -> p t b", t=tc),
                        xT[ch * tc:(ch + 1) * tc].rearrange("t p b -> p t b"),
                    )
                chunk_tiles[ch] = t
            return chunk_tiles[ch]

        cur = get_chunk(0)
        # hh_{-1} = 0
        nc.vector.memset(cur[0:H, 0:B], 0.0)

        c_prev = []
        for blk in range(nb):
            c0 = cpool.tile([3 * H, fd], F32, name=f"cc{blk}", tag=f"cc{blk}")
            nc.vector.memset(c0[2 * H:3 * H], 0.0)
            c_prev.append(c0)

        # Phase A(b, s): mm -> sigma_all -> t1 -> u ; v (Pool)
        # Phase B(b, s): cc -> ts -> hh
        state = {}

        def phase_a(blk, s_global):
            ch_, s_ = divmod(s_global, tc)
            col = s_ * B + blk * fd
            rhs = get_chunk(ch_)
            p = pspool.tile([G, fd], F32, name="gates", tag=f"gates{blk}")
            _mm_noldw(nc, p[:], w_t[:], rhs[:, col:col + fd])
            # S = sigmoid(z): Sg2@0, Si@32, Sf@64, So@96
            s_t = spool.tile([G, fd], F32, name="sgm", tag=f"sgm{blk}")
            nc.scalar.activation(s_t[:], p[:], SIG, bias=bg_t[:])
            # t1 = S_g - 0.5 relocated to start 32 (pairs with i)
            t1 = tpool.tile([2 * H, fd], F32, name="t1", tag=f"t1{blk}")
            nc.vector.tensor_scalar(t1[H:2 * H], s_t[0:H], 0.5, None, SUB)
            # v = f * cc_prev at start 64 (Pool, off the DVE chain)
            v = tpool.tile([3 * H, fd], F32, name="v", tag=f"v{blk}")
            nc.gpsimd.tensor_tensor(
                v[2 * H:3 * H], s_t[2 * H:3 * H], c_prev[blk][2 * H:3 * H], MULT)
            state[blk] = (s_t, t1, v, s_global)

        def phase_b(blk):
            s_t, t1, v, s_global = state[blk]
            # u = t1 * S_i issued a half later than t1, avoiding the DVE
            # write->read pipeline stall observed when they run back-to-back
            u = tpool.tile([3 * H, fd], F32, name="u", tag=f"u{blk}")
            nc.vector.tensor_tensor(u[2 * H:3 * H], t1[H:2 * H], s_t[H:2 * H],
                                    MULT)
            c_new = cpool.tile([3 * H, fd], F32, name=f"ccn{blk}",
                               tag=f"cc{blk}")
            nc.vector.tensor_tensor(c_new[2 * H:3 * H], u[2 * H:3 * H],
                                    v[2 * H:3 * H], ADD)
            c_prev[blk] = c_new
            # ts = sigmoid(4*cc) relocated to start 96 (pairs with o)
            ts = taupool.tile([G, fd], F32, name="ts", tag=f"ts{blk}")
            nc.scalar.activation(ts[3 * H:4 * H], c_new[2 * H:3 * H],
                                 SIG, scale=4.0)
            ch_, s_ = divmod(s_global + 1, tc)
            col = s_ * B + blk * fd
            hdst = get_chunk(ch_)[0:H, col:col + fd]
            nc.vector.scalar_tensor_tensor(
                hdst, ts[3 * H:4 * H], 0.5, s_t[3 * H:4 * H], SUB, MULT)

        def emit_out(ch):
            cur_, nxt_ = get_chunk(ch), get_chunk(ch + 1)
            nc.sync.dma_start(
                hout[ch * tc:ch * tc + tc - 1].rearrange("t p b -> p t b"),
                cur_[0:H, B:].rearrange("p (t b) -> p t b", t=tc - 1),
            )
            nc.sync.dma_start(hout[ch * tc + tc - 1], nxt_[0:H, 0:B])

        def tick(n):
            tc_.tile_set_cur_wait(n * HALF_NS / 1e6)

        # Half h: [B-phase of one block, then A-phase of the other]. Issuing B
        # first keeps each block's DVE chain ops from queueing behind the
        # other block's, so the two data cycles overlap cleanly.
        tick(0)
        phase_a(0, 0)
        for s in range(t_total):
            tick(2 * s + 1)
            phase_b(0)
            phase_a(1, s)
            tick(2 * s + 2)
            phase_b(1)
            if s + 1 < t_total:
                phase_a(0, s + 1)
            if s % tc == tc - 1:
                emit_out(s // tc)
    nc.compile()
    return nc


def _prep_weights(W_emb, b_emb, W_ih, W_hh, b_ih, b_hh):
    f8 = lambda a: np.asarray(a, np.float64)
    Wx = f8(W_ih) @ f8(W_emb)                                  # [4H, IN]
    bgv = f8(W_ih) @ f8(b_emb) + f8(b_ih) + f8(b_hh)           # [4H]
    perm = np.r_[2 * H:3 * H, 0:H, H:2 * H, 3 * H:4 * H]       # [g,i,f,o]
    wc = np.concatenate([f8(W_hh)[perm].T, Wx[perm].T], axis=0)  # [K, G]
    wc[0:H, :] *= 2.0           # hh-rows: rhs holds h/2
    wc[:, 0:H] *= 2.0           # g-gate columns: sigmoid(2 z_g)
    bgv = bgv[perm].copy()
    bgv[0:H] *= 2.0
    return (np.ascontiguousarray(wc.astype(ml_dtypes.bfloat16)),
            np.ascontiguousarray(bgv.astype(np.float32).reshape(G, 1)))


def _run(x, W_emb, b_emb, W_ih, W_hh, b_ih, b_hh, trace=False):
    t_total = x.shape[0]
    key = (t_total, TC, NB)
    if key not in _CACHE:
        _CACHE[key] = _build(t_total, TC, NB)
    nc = _CACHE[key]

    wc, bgv = _prep_weights(W_emb, b_emb, W_ih, W_hh, b_ih, b_hh)
    x = np.asarray(x, np.float32)
    in_maps = []
    for c in range(NCORES):
        xs = np.ascontiguousarray(
            x[:, c * B:(c + 1) * B, :].transpose(0, 2, 1)).astype(
                ml_dtypes.bfloat16)  # [T, IN, B] bf16
        in_maps.append({"xT": xs, "wcat": wc, "bg": bgv})

    res = run_bass_kernel_spmd(nc, in_maps, list(range(NCORES)), trace=trace)
    out = np.empty((t_total, B_FULL, H), np.float32)
    for c in range(NCORES):
        out[:, c * B:(c + 1) * B, :] = np.asarray(
            res.results[c]["hout"], np.float32).transpose(0, 2, 1) * 2.0
    return out, res


def kernel(x, W_emb, b_emb, W_ih, W_hh, b_ih, b_hh):
    out, _ = _run(x, W_emb, b_emb, W_ih, W_hh, b_ih, b_hh, trace=False)
    return out
